# revision 2
# baseline (speedup 1.0000x reference)
"""Trainium2 Bass kernel for nn_Caption (LSTM caption decoder).

Distribution: pure data-parallel over batch (128 -> 8 cores x 16), no
collectives. Per core: x0 projection, embedding gather, input-gate GEMM,
40-step LSTM recurrence, vocab GEMM.

Layout strategy (H-layout): everything keeps the feature dim on partitions
and the tiny per-core batch (16) on the free dim, so recurrence matmuls
stream only 16 columns each and no transposes are needed anywhere after
the initial embedding transpose:
  - seqT  [128, 4, 640]  : input embeddings, e on partitions, (t,b) cols
  - xg    [128, 40, 256] : precomputed W_ih @ seq + bias, t-major, per step
                           cols = [slot(i,f,o,g) x chunk(4) x b(16)]
  - gates PSUM [128, 4, 64]: slot-major, injected from xg via identity
                           matmul, accumulated with W_hh @ h_prev
  - hidT  [128, 4, 640]  : h states, h-dim on partitions -> feeds both the
                           next step's matmuls and the vocab GEMM directly
  - vocab GEMM: out [128 vocab, cols] in two sweeps (cols 0:256 interleaved
                into steps 16..39; cols 256:640 in the tail), W_out streamed
                twice from HBM, result staged bf16 and DMA'd per vt-group.
"""
import sys

sys.path.insert(0, "/opt/trn_rl_repo")

import numpy as np
import ml_dtypes

import concourse.bass as bass
import concourse.tile as tile
from concourse import bacc, mybir
from concourse.bass_utils import run_bass_kernel_spmd

BF = mybir.dt.bfloat16
F32 = mybir.dt.float32
I32 = mybir.dt.int32
bfnp = ml_dtypes.bfloat16

B, F, E, H, V, T = 128, 1536, 512, 512, 10000, 40
NCORES = 8
BC = B // NCORES          # 16 batch rows per core
NT = T * BC               # 640 (t,b) columns
G4 = 4 * H                # 2048 gate dims (torch order i,f,g,o)
NGT = 16                  # gate tiles of 128
VP = 10240                # padded vocab
NVT = VP // 128           # 80 vocab tiles
NVG = NVT // 4            # 20 vocab weight groups (4 tiles each)
SLOT_OF_GATE = [0, 1, 3, 2]   # torch gate (i,f,g,o) -> psum slot (i,f,o,g)

# interleave knobs
XG_PER_STEP = 6           # xg units emitted per step (t < 16)
VOC_PER_STEP = 2          # sweep-A vocab units per step (t >= 16)

_CACHE = {}


def _build():
    if "nc" in _CACHE:
        return _CACHE["nc"]
    nc = bacc.Bacc("TRN2", target_bir_lowering=False, debug=False,
                   num_devices=NCORES)

    featT_d = nc.dram_tensor("featT", [128, 12, BC], BF, kind="ExternalInput")
    idx_d = nc.dram_tensor("idx", [NT, 1], I32, kind="ExternalInput")
    emb_d = nc.dram_tensor("embt", [V, E], BF, kind="ExternalInput")
    WinT_d = nc.dram_tensor("WinT", [128, 12, E], BF, kind="ExternalInput")
    WihT_d = nc.dram_tensor("WihT", [128, 4, G4], BF, kind="ExternalInput")
    WhhT_d = nc.dram_tensor("WhhT", [128, 4, G4], BF, kind="ExternalInput")
    bvec_d = nc.dram_tensor("bvec", [128, NGT], F32, kind="ExternalInput")
    binT_d = nc.dram_tensor("binT", [128, 4], F32, kind="ExternalInput")
    ident_d = nc.dram_tensor("ident", [128, 128], BF, kind="ExternalInput")
    wout_d = nc.dram_tensor("wout", [NVG, 128, 2048], BF,
                            kind="ExternalInput")
    out_d = nc.dram_tensor("out_q", [NVT, 128, NT], BF, kind="ExternalOutput")

    with tile.TileContext(nc) as tc:
        with (
            tc.tile_pool(name="consts", bufs=1) as consts,
            tc.tile_pool(name="big", bufs=1) as big,
            tc.tile_pool(name="state", bufs=2) as state,
            tc.tile_pool(name="work", bufs=2) as work,
            tc.tile_pool(name="wpool", bufs=3) as wpool,
        ):
            # ---- constants / weights ----
            idx_sb = consts.tile([128, 5, 1], I32)
            nc.sync.dma_start(
                idx_sb[:], idx_d.ap().rearrange("(j p) o -> p j o", p=128))
            identb = consts.tile([128, 128], BF)
            nc.sync.dma_start(identb[:], ident_d.ap())
            featT_sb = consts.tile([128, 12, BC], BF)
            nc.sync.dma_start(featT_sb[:], featT_d.ap())
            binT_sb = consts.tile([128, 4], F32)
            nc.sync.dma_start(binT_sb[:], binT_d.ap())
            bvec_sb = consts.tile([128, NGT], F32)
            nc.sync.dma_start(bvec_sb[:], bvec_d.ap())
            WihT_sb = big.tile([128, 4, G4], BF, tag="wih")
            nc.sync.dma_start(WihT_sb[:], WihT_d.ap())
            WhhT_sb = big.tile([128, 4, G4], BF, tag="whh")
            nc.sync.dma_start(WhhT_sb[:], WhhT_d.ap())
            WinT_sb = big.tile([128, 12, E], BF, tag="win")
            nc.sync.dma_start(WinT_sb[:], WinT_d.ap())

            seqT = big.tile([128, 4, NT], BF, tag="seqT")
            xg_sb = big.tile([128, T, 256], BF, tag="xg")
            hidT = big.tile([128, 4, NT], BF, tag="hidT")
            stage = big.tile([128, NVT, NT], BF, tag="stage")

            # ---- embedding gather -> seqT (transposed via PE) ----
            with tc.tile_pool(name="psA", bufs=3, space="PSUM") as psA:
                for j in range(5):
                    gt = work.tile([128, E], BF, tag="gather")
                    nc.gpsimd.indirect_dma_start(
                        out=gt[:], out_offset=None, in_=emb_d.ap(),
                        in_offset=bass.IndirectOffsetOnAxis(
                            ap=idx_sb[:, j, :], axis=0))
                    for e in range(4):
                        pst = psA.tile([128, 128], BF, space="PSUM", tag="tr")
                        nc.tensor.transpose(
                            pst[:], gt[:, e * 128:(e + 1) * 128], identb[:])
                        if j == 0:
                            # cols 0:16 belong to x0 (written below)
                            nc.vector.tensor_copy(
                                seqT[:, e, BC:128], pst[:, BC:128])
                        else:
                            nc.vector.tensor_copy(
                                seqT[:, e, j * 128:(j + 1) * 128], pst[:])

                # ---- x0T = W_inT.T @ featT + b_in -> seqT[:, :, 0:BC] ----
                for m in range(4):
                    ps = psA.tile([128, BC], F32, space="PSUM", tag="x0")
                    for k in range(12):
                        nc.tensor.matmul(
                            ps[:], lhsT=WinT_sb[:, k, m * 128:(m + 1) * 128],
                            rhs=featT_sb[:, k, :],
                            start=(k == 0), stop=(k == 11))
                    nc.scalar.activation(
                        seqT[:, m, 0:BC], ps[:],
                        mybir.ActivationFunctionType.Identity,
                        bias=binT_sb[:, m:m + 1])

            # ---- main phase pools: gates (2 banks) + units (5 banks) ----
            gates_pool = tc.tile_pool(name="psGates", bufs=2, space="PSUM")
            units_pool = tc.tile_pool(name="psUnits", bufs=5, space="PSUM")
            gpsum = gates_pool.__enter__()
            upsum = units_pool.__enter__()

            ncopy = [0]

            def copy_out(dst, src, bias_ap=None):
                """Alternate PSUM->SBUF copies between ACT and DVE."""
                ncopy[0] += 1
                if bias_ap is not None:
                    nc.vector.tensor_scalar_add(dst, src, bias_ap)
                elif ncopy[0] % 2 == 0:
                    nc.scalar.copy(dst, src)
                else:
                    nc.vector.tensor_copy(dst, src)

            def emit_xg_unit(gt, tch):
                """xg for torch gate-tile gt, timestep chunk tch (8 steps)."""
                ps = upsum.tile([128, 128], F32, space="PSUM", tag="u",
                                name="xps")
                cols = slice(tch * 128, (tch + 1) * 128)
                for k in range(4):
                    nc.tensor.matmul(
                        ps[:], lhsT=WihT_sb[:, k, gt * 128:(gt + 1) * 128],
                        rhs=seqT[:, k, cols],
                        start=(k == 0), stop=(k == 3))
                slot, ch = SLOT_OF_GATE[gt // 4], gt % 4
                # write [128, 8, 16] strided into xg_sb[:, t0:t0+8, off:off+16]
                dst = xg_sb[:, tch * 8:(tch + 1) * 8,
                            slot * 64 + ch * 16:slot * 64 + (ch + 1) * 16]
                src = ps[:].rearrange("p (t b) -> p t b", t=8)
                nc.vector.tensor_scalar_add(dst, src, bvec_sb[:, gt:gt + 1])

            def emit_vocab_unit(vt, sweep):
                """vocab tile vt for sweep 0 (cols 0:256) / 1 (cols 256:640)."""
                c0, c1 = (0, 256) if sweep == 0 else (256, NT)
                nbc = c1 - c0
                g, vt4 = vt // 4, vt % 4
                wt = wtiles[g]
                ps = upsum.tile([128, 384], F32, space="PSUM", tag="u",
                                name="vps")
                for k in range(4):
                    nc.tensor.matmul(
                        ps[:, 0:nbc],
                        lhsT=wt[:, vt4, k, :],
                        rhs=hidT[:, k, c0:c1],
                        start=(k == 0), stop=(k == 3))
                copy_out(stage[:, vt, c0:c1], ps[:, 0:nbc])

            wtiles = {}

            def fetch_wgroup(g, sweep):
                wt = wpool.tile([128, 4, 4, 128], BF, tag="wout",
                                name=f"wt{sweep}_{g}")
                nc.sync.dma_start(
                    wt[:], wout_d.ap()[g].rearrange(
                        "p (v k j) -> p v k j", v=4, k=4))
                wtiles[g] = wt

            # preamble xg: timestep chunk 0 for all 16 gate tiles
            for gt in range(NGT):
                emit_xg_unit(gt, 0)

            xg_units = [(gt, tch) for tch in range(1, 5) for gt in range(NGT)]
            vocA_units = [(vt, 0) for vt in range(NVT)]
            vocB_units = [(vt, 1) for vt in range(NVT)]

            # ---- LSTM recurrence with interleaved xg / vocab units ----
            c_prev = None
            for t in range(T):
                gates = gpsum.tile([128, 4, 64], F32, space="PSUM",
                                   tag="gates", name="gates")
                # inject xg + accumulate W_hh @ h_prev, per (slot, chunk)
                for s in range(4):
                    for ch in range(4):
                        gsl = gates[:, s, ch * 16:(ch + 1) * 16]
                        xsl = xg_sb[:, t,
                                    s * 64 + ch * 16:s * 64 + (ch + 1) * 16]
                        nc.tensor.matmul(
                            gsl, lhsT=identb[:], rhs=xsl,
                            start=True, stop=(t == 0),
                            skip_group_check=True)
                        if t > 0:
                            gt_torch = [0, 1, 3, 2][s] * 4 + ch
                            for k in range(4):
                                nc.tensor.matmul(
                                    gsl,
                                    lhsT=WhhT_sb[:, k, gt_torch * 128:
                                                 (gt_torch + 1) * 128],
                                    rhs=hidT[:, k, (t - 1) * BC:t * BC],
                                    start=False, stop=(k == 3),
                                    skip_group_check=True)

                sig = state.tile([128, 3, 64], F32, tag="sig")
                g_t = state.tile([128, 64], F32, tag="g")
                nc.scalar.activation(
                    sig[:], gates[:, 0:3, :],
                    mybir.ActivationFunctionType.Sigmoid)
                nc.scalar.activation(
                    g_t[:], gates[:, 3, :],
                    mybir.ActivationFunctionType.Tanh)

                c_new = state.tile([128, 64], F32, tag="c")
                if t == 0:
                    nc.vector.tensor_mul(c_new[:], sig[:, 0, :], g_t[:])
                else:
                    ig = state.tile([128, 64], F32, tag="ig")
                    nc.vector.tensor_mul(ig[:], sig[:, 0, :], g_t[:])
                    cf = state.tile([128, 64], F32, tag="cf")
                    nc.vector.tensor_mul(cf[:], sig[:, 1, :], c_prev[:])
                    nc.vector.tensor_add(c_new[:], ig[:], cf[:])
                c_prev = c_new
                tc_t = state.tile([128, 64], F32, tag="tanhc")
                nc.scalar.activation(
                    tc_t[:], c_new[:], mybir.ActivationFunctionType.Tanh)
                hdst = hidT[:, :, t * BC:(t + 1) * BC]
                tsrc = tc_t[:].rearrange("p (k b) -> p k b", k=4)
                osrc = sig[:, 2, :].rearrange("p (k b) -> p k b", k=4)
                nc.vector.tensor_mul(hdst, osrc, tsrc)

                # interleaved filler work
                if t < 16:
                    for _ in range(XG_PER_STEP):
                        if xg_units:
                            emit_xg_unit(*xg_units.pop(0))
                else:
                    if t == 16:
                        fetch_wgroup(0, 0)
                        fetch_wgroup(1, 0)
                    for _ in range(VOC_PER_STEP):
                        if vocA_units:
                            vt, sw = vocA_units.pop(0)
                            if vt % 4 == 0 and vt // 4 + 2 < NVG:
                                fetch_wgroup(vt // 4 + 2, 0)
                            emit_vocab_unit(vt, sw)

            # ---- tail: finish sweep A, then sweep B with grouped out-DMA ----
            while vocA_units:
                vt, sw = vocA_units.pop(0)
                if vt % 4 == 0 and vt // 4 + 2 < NVG:
                    fetch_wgroup(vt // 4 + 2, 0)
                emit_vocab_unit(vt, sw)

            fetch_wgroup(0, 1)
            fetch_wgroup(1, 1)
            while vocB_units:
                vt, sw = vocB_units.pop(0)
                if vt % 4 == 0 and vt // 4 + 2 < NVG:
                    fetch_wgroup(vt // 4 + 2, 1)
                emit_vocab_unit(vt, sw)
                if vt % 8 == 7:
                    g8 = vt // 8
                    nc.gpsimd.dma_start(
                        out_d.ap()[g8 * 8:(g8 + 1) * 8].rearrange(
                            "v p c -> p v c"),
                        stage[:, g8 * 8:(g8 + 1) * 8, :])

            units_pool.__exit__(None, None, None)
            gates_pool.__exit__(None, None, None)

    nc.compile()
    _CACHE["nc"] = nc
    return nc


def kernel(features, seqs, lengths, W_in, b_in, emb, W_ih, W_hh, b_ih, b_hh,
           W_out, b_out):
    f32 = lambda x: np.asarray(x, dtype=np.float32)
    bf = lambda x: np.ascontiguousarray(f32(x)).astype(bfnp)
    features = f32(features)
    seqs = np.asarray(seqs).astype(np.int64)

    WinT = np.ascontiguousarray(
        bf(f32(W_in).T).reshape(12, 128, E).transpose(1, 0, 2))
    WihT = np.ascontiguousarray(
        bf(f32(W_ih).T).reshape(4, 128, G4).transpose(1, 0, 2))
    WhhT = np.ascontiguousarray(
        bf(f32(W_hh).T).reshape(4, 128, G4).transpose(1, 0, 2))
    bvec = np.ascontiguousarray(
        (f32(b_ih) + f32(b_hh)).reshape(NGT, 128).T)        # [128, 16]
    binT = np.ascontiguousarray(f32(b_in).reshape(4, 128).T)  # [128, 4]
    emb_b = bf(emb)
    WoutT = np.zeros((H, VP), dtype=bfnp)
    WoutT[:, :V] = bf(f32(W_out).T)
    # [g, p, (vt4 k j)]: element = WoutT[k*128+p, (4g+vt4)*128+j]
    wout = np.ascontiguousarray(
        WoutT.reshape(4, 128, NVG, 4, 128)
        .transpose(2, 1, 3, 0, 4).reshape(NVG, 128, 2048))
    ident_np = np.eye(128, dtype=bfnp)

    nc = _build()
    in_maps = []
    for c in range(NCORES):
        bs = slice(c * BC, (c + 1) * BC)
        featT = np.ascontiguousarray(
            bf(features[bs].T).reshape(12, 128, BC).transpose(1, 0, 2))
        tok = np.zeros((T, BC), np.int64)
        tok[1:] = seqs[bs].T                     # t-major, t=0 dummy
        in_maps.append({
            "featT": featT,
            "idx": tok.reshape(NT, 1).astype(np.int32),
            "embt": emb_b,
            "WinT": WinT, "WihT": WihT, "WhhT": WhhT,
            "bvec": bvec, "binT": binT, "ident": ident_np,
            "wout": wout,
        })
    _CACHE["last_in_maps"] = in_maps
    res = run_bass_kernel_spmd(nc, in_maps, list(range(NCORES)))
    out = np.empty((B, T, V), np.float32)
    for c in range(NCORES):
        oq = np.asarray(res.results[c]["out_q"]).astype(np.float32)
        lt = oq.reshape(VP, NT)[:V]              # [V, 640]
        out[c * BC:(c + 1) * BC] = lt.reshape(V, T, BC).transpose(2, 1, 0)
    bo = f32(b_out)
    if np.any(bo):
        out += bo
    return out
